# revision 17
# baseline (speedup 1.0000x reference)
"""Trainium2 Bass kernel for quantized Conformer MHSA (nn_ConformerMHSAWithGateV1).

Sharding: data-parallel over batch B=32 across 8 cores (4 batches/core).
All per-tensor fake-quant scales are global -> 6 tiny AllReduces (max/min).

Numerics strategy:
  - fq() produces integer grids |v|<=128 -> bf16 matmul operands are EXACT.
  - round() via fp32 magic-constant trick (RNE, matches jnp.round).
  - softmax: max(attn_row) == 1/Z_row exactly, so the global fq(attn) scale
    needs only AllReduce-min(Z); pass-2 recomputes scores transposed with the
    per-row bias -(m + ln(Z*s_a)/alpha) folded in via a K=1 matmul, so the
    ACT exp directly emits attn/s_a ready for one-op rounding.
"""
import sys

sys.path.insert(0, "/opt/trn_rl_repo")

import numpy as np
import ml_dtypes

import concourse.bass as bass
import concourse.mybir as mybir
import concourse.tile as tile
from concourse import bacc, bass_isa
from concourse.bass_utils import run_bass_kernel_spmd
from concourse.masks import make_identity

F32 = mybir.dt.float32
BF16 = mybir.dt.bfloat16
AX = mybir.AxisListType
OP = mybir.AluOpType
AF = mybir.ActivationFunctionType

B, T, F, H, DK = 32, 512, 512, 8, 64
NCORES = 8
BC = B // NCORES          # batches per core
TC = BC * T               # token rows per core
P = 128
QMAX = 127.0
LN_EPS = 1e-5
MAGIC_S = 12582912.0      # 1.5*2^23: round-to-int for signed fp32 in [-2^22, 2^22]
MAGIC_U = 8388608.0       # 2^23:     round-to-int for fp32 in [0, 2^23)
RG = [list(range(NCORES))]

_CACHE = {}


def _build(dbg=False):
    import contextlib

    nc = bacc.Bacc(None, target_bir_lowering=False, debug=False)

    x_in = nc.dram_tensor("x", [TC, F], F32, kind="ExternalInput")
    w_in = {n: nc.dram_tensor(n, [F, F], BF16, kind="ExternalInput")
            for n in ("wq", "wk", "wv", "wo")}
    b_in = {n: nc.dram_tensor(n, [F], F32, kind="ExternalInput")
            for n in ("bq", "bk", "bv", "bo")}
    wscl_in = nc.dram_tensor("wscl", [1, 4], F32, kind="ExternalInput")
    out_d = nc.dram_tensor("out", [TC, F], F32, kind="ExternalOutput")
    if dbg:
        dbg_scl = nc.dram_tensor("dbg_scl", [1, 8], F32, kind="ExternalOutput")
        dbg_xi = nc.dram_tensor("dbg_xi", [P, 4 * TC], BF16, kind="ExternalOutput")
        dbg_qi = nc.dram_tensor("dbg_qi", [P, 4 * TC], BF16, kind="ExternalOutput")
        dbg_ki = nc.dram_tensor("dbg_ki", [P, 4 * TC], BF16, kind="ExternalOutput")
        dbg_mz = nc.dram_tensor("dbg_mz", [P, 2 * P], F32, kind="ExternalOutput")
        dbg_ai = nc.dram_tensor("dbg_ai", [P, 4 * 512], BF16, kind="ExternalOutput")
        dbg_ci = nc.dram_tensor("dbg_ci", [P, 4 * TC], BF16, kind="ExternalOutput")

    with tile.TileContext(nc) as tc, contextlib.ExitStack() as stack:
        const = stack.enter_context(tc.tile_pool(name="const", bufs=1))
        tiny = stack.enter_context(tc.tile_pool(name="tiny", bufs=1))
        dram = stack.enter_context(tc.tile_pool(name="dram", bufs=1, space="DRAM"))
        persist = stack.enter_context(tc.tile_pool(name="persist", bufs=1))

        # ---- constants / params ----
        ident = const.tile([P, P], F32, name="ident")
        make_identity(nc, ident)
        neg1 = const.tile([1, P], F32, name="neg1")
        nc.gpsimd.memset(neg1[:], -1.0)
        nmag_b = const.tile([P, 1], F32, name="nmag_b")
        nc.gpsimd.memset(nmag_b[:], -MAGIC_S)
        w_sb = {}
        for n in ("wq", "wk", "wv", "wo"):
            w_sb[n] = const.tile([P, 4, F], BF16, name=f"{n}_sb")
            nc.sync.dma_start(w_sb[n][:], w_in[n].ap().rearrange("(c p) f -> p c f", p=P))
        bq_sb = const.tile([P, 4], F32, name="bq_sb")
        bk_sb = const.tile([P, 4], F32, name="bk_sb")
        nc.sync.dma_start(bq_sb[:], b_in["bq"].ap().rearrange("(a p) -> p a", p=P))
        nc.sync.dma_start(bk_sb[:], b_in["bk"].ap().rearrange("(a p) -> p a", p=P))
        bv_rep = const.tile([P, F], F32, name="bv_rep")
        bo_rep = const.tile([P, F], F32, name="bo_rep")
        nc.sync.dma_start(bv_rep[:1, :], b_in["bv"].ap().rearrange("(o f) -> o f", o=1))
        nc.gpsimd.partition_broadcast(bv_rep[:], bv_rep[:1, :])
        nc.sync.dma_start(bo_rep[:1, :], b_in["bo"].ap().rearrange("(o f) -> o f", o=1))
        nc.gpsimd.partition_broadcast(bo_rep[:], bo_rep[:1, :])
        wscl = tiny.tile([1, 4], F32, name="wscl")
        nc.sync.dma_start(wscl[:], wscl_in.ap())

        def bcast(src11, name):
            t = tiny.tile([P, 1], F32, name=name)
            nc.gpsimd.partition_broadcast(t[:], src11)
            return t

        def cross_part_max(vec, name):
            r = tiny.tile([P, 1], F32, name=name)
            nc.gpsimd.partition_all_reduce(r[:], vec, channels=P,
                                           reduce_op=bass_isa.ReduceOp.max)
            return r

        def all_reduce(src, n, op, name):
            ci = dram.tile([1, n], F32, name=f"cci_{name}")
            co = dram.tile([1, n], F32, addr_space="Shared", name=f"cco_{name}")
            nc.sync.dma_start(ci[:], src)
            nc.gpsimd.collective_compute(
                "AllReduce", op, replica_groups=RG,
                ins=[ci[:].opt()], outs=[co[:].opt()])
            r = tiny.tile([1, n], F32, name=f"ar_{name}")
            nc.sync.dma_start(r[:], co[:])
            return r

        def scale_of(armax, name):
            s = tiny.tile([1, 1], F32, name=f"s_{name}")
            nc.vector.tensor_scalar(s[:], armax, 1.0 / QMAX, 1e-8, OP.mult, OP.max)
            r = tiny.tile([1, 1], F32, name=f"r_{name}")
            nc.vector.reciprocal(r[:], s[:])
            return s, r

        # ---- persistent int activations ----
        xT = persist.tile([P, 4, TC], BF16, name="xT")       # x_int^T [fi, tok]
        qiT = persist.tile([P, 4, TC], BF16, name="qiT")     # q_int^T [fo, tok]
        kiT = persist.tile([P, 4, TC], BF16, name="kiT")
        vi = persist.tile([P, 16, F], BF16, name="vi")       # v_int   [tok, fo]

        # ============ stages 1-2: layernorm, fq(x), transpose ============
        with (
            tc.tile_pool(name="early_a", bufs=1) as early_a,
            tc.tile_pool(name="scr_a", bufs=2) as scr_a,
            tc.tile_pool(name="tp_ps", bufs=4, space="PSUM") as tp_ps,
        ):
            x_sb = early_a.tile([P, 16, F], F32, name="x_sb")
            nc.sync.dma_start(x_sb[:], x_in.ap().rearrange("(n p) f -> p n f", p=P))
            mu = early_a.tile([P, 16], F32, name="mu")
            ssq = early_a.tile([P, 16], F32, name="ssq")
            rmaxt = early_a.tile([P, 16], F32, name="rmaxt")
            for i in range(16):
                s1 = scr_a.tile([P, F], F32, name="lnscr")
                nc.scalar.activation(s1[:], x_sb[:, i], AF.Copy, bias=0.0,
                                     scale=1.0 / F, accum_out=mu[:, i:i + 1])
                nc.vector.tensor_scalar(x_sb[:, i], x_sb[:, i], mu[:, i:i + 1],
                                        None, OP.subtract)
                s2 = scr_a.tile([P, F], F32, name="lnscr")
                nc.scalar.activation(s2[:], x_sb[:, i], AF.Square, bias=0.0,
                                     scale=1.0, accum_out=ssq[:, i:i + 1])
                nc.vector.tensor_reduce(rmaxt[:, i:i + 1], x_sb[:, i], axis=AX.X,
                                        op=OP.max, apply_absolute_value=True)
            var = early_a.tile([P, 16], F32, name="var")
            nc.vector.tensor_scalar(var[:], ssq[:], 1.0 / F, LN_EPS, OP.mult, OP.add)
            stdv = early_a.tile([P, 16], F32, name="stdv")
            nc.scalar.activation(stdv[:], var[:], AF.Sqrt, bias=0.0, scale=1.0)
            rstd = early_a.tile([P, 16], F32, name="rstd")
            nc.vector.reciprocal(rstd[:], stdv[:])
            # absmax of LN output = rstd * rowmax|x-mu| (monotone => exact)
            gx = early_a.tile([P, 16], F32, name="gx")
            nc.vector.tensor_tensor(gx[:], rstd[:], rmaxt[:], OP.mult)
            gx1 = early_a.tile([P, 1], F32, name="gx1")
            nc.vector.tensor_reduce(gx1[:], gx[:], axis=AX.X, op=OP.max)
            gxm = cross_part_max(gx1[:], "gxm")
            ar1 = all_reduce(gxm[:1, :], 1, OP.max, "sx")
            s_x, r_x = scale_of(ar1[:1, :], "x")
            r_xb = bcast(r_x[:1, :], "r_xb")
            rstd_r = early_a.tile([P, 16], F32, name="rstd_r")
            nc.vector.tensor_scalar(rstd_r[:], rstd[:], r_xb[:, :1], None, OP.mult)

            # x_int (+MAGIC), PE transpose, evac with -MAGIC -> xT bf16
            for i in range(16):
                u = scr_a.tile([P, F], F32, name="uq")
                nc.vector.tensor_scalar(u[:], x_sb[:, i], rstd_r[:, i:i + 1],
                                        MAGIC_S, OP.mult, OP.add)
                for j in range(4):
                    pt = tp_ps.tile([P, P], F32, name="tpp")
                    nc.tensor.transpose(pt[:], u[:, j * P:(j + 1) * P], ident[:])
                    nc.scalar.activation(xT[:, j, i * P:(i + 1) * P], pt[:],
                                         AF.Identity, bias=nmag_b[:, :1], scale=1.0)
            if dbg:
                nc.sync.dma_start(dbg_xi.ap().rearrange("p (c t) -> p c t", c=4), xT[:])

        # ============ stage 3a: q,k projections, AR2qk, fq(q), fq(k) ============
        sc_q = tiny.tile([1, 1], F32, name="sc_q")
        sc_k = tiny.tile([1, 1], F32, name="sc_k")
        sc_v = tiny.tile([1, 1], F32, name="sc_v")
        nc.vector.tensor_tensor(sc_q[:], s_x[:], wscl[:1, 0:1], OP.mult)
        nc.vector.tensor_tensor(sc_k[:], s_x[:], wscl[:1, 1:2], OP.mult)
        nc.vector.tensor_tensor(sc_v[:], s_x[:], wscl[:1, 2:3], OP.mult)
        scb_q = bcast(sc_q[:1, :], "scb_q")
        scb_k = bcast(sc_k[:1, :], "scb_k")
        scb_v = bcast(sc_v[:1, :], "scb_v")

        with (
            tc.tile_pool(name="early_b", bufs=1) as early_b,
            tc.tile_pool(name="scr_b", bufs=2) as scr_b,
            tc.tile_pool(name="mm_ps", bufs=4, space="PSUM") as mm_ps,
        ):
            qT = early_b.tile([P, 4, TC], F32, name="qT")
            kT = early_b.tile([P, 4, TC], F32, name="kT")
            for wname, dstT, bsb, scb in (("wq", qT, bq_sb, scb_q),
                                          ("wk", kT, bk_sb, scb_k)):
                for a in range(4):
                    for n in range(4):
                        ps = mm_ps.tile([P, 512], F32, name="qkps")
                        for c in range(4):
                            nc.tensor.matmul(
                                ps[:], lhsT=w_sb[wname][:, c, a * P:(a + 1) * P],
                                rhs=xT[:, c, n * 512:(n + 1) * 512],
                                start=(c == 0), stop=(c == 3))
                        nc.scalar.activation(dstT[:, a, n * 512:(n + 1) * 512],
                                             ps[:], AF.Identity,
                                             bias=bsb[:, a:a + 1], scale=scb[:, :1])
            vals2 = tiny.tile([1, 2], F32, name="vals2")
            for idx, src in enumerate((qT, kT)):
                r1 = tiny.tile([P, 1], F32, name=f"qkmax{idx}")
                nc.vector.tensor_reduce(r1[:], src[:], axis=AX.XY, op=OP.max,
                                        apply_absolute_value=True)
                rm = cross_part_max(r1[:], f"qkgm{idx}")
                nc.vector.tensor_copy(vals2[:1, idx:idx + 1], rm[:1, :])
            ar2 = all_reduce(vals2[:1, :], 2, OP.max, "qk")
            s2t = tiny.tile([1, 2], F32, name="s2t")
            nc.vector.tensor_scalar(s2t[:], ar2[:], 1.0 / QMAX, 1e-8, OP.mult, OP.max)
            r2t = tiny.tile([1, 2], F32, name="r2t")
            nc.vector.reciprocal(r2t[:], s2t[:])
            r_qb = bcast(r2t[:1, 0:1], "r_qb")
            r_kb = bcast(r2t[:1, 1:2], "r_kb")
            # alpha = s_q*s_k/8  (scores scale; /sqrt(64) == /8 exact)
            alpha = tiny.tile([1, 1], F32, name="alpha")
            nc.vector.tensor_tensor(alpha[:], s2t[:1, 0:1], s2t[:1, 1:2], OP.mult)
            nc.vector.tensor_scalar(alpha[:], alpha[:], 0.125, None, OP.mult)
            nalpha = tiny.tile([1, 1], F32, name="nalpha")
            nc.vector.tensor_scalar(nalpha[:], alpha[:], -1.0, None, OP.mult)
            ralpha = tiny.tile([1, 1], F32, name="ralpha")
            nc.vector.reciprocal(ralpha[:], alpha[:])
            alb = bcast(alpha[:1, :], "alb")
            nalb = bcast(nalpha[:1, :], "nalb")
            rab = bcast(ralpha[:1, :], "rab")

            for src, dst, rb in ((qT, qiT, r_qb), (kT, kiT, r_kb)):
                for c in range(4):
                    tq = scr_b.tile([P, TC], F32, name="tqnt")
                    nc.vector.tensor_scalar(tq[:], src[:, c], rb[:, :1], MAGIC_S,
                                            OP.mult, OP.add)
                    nc.vector.tensor_scalar(dst[:, c], tq[:], MAGIC_S, None,
                                            OP.subtract)
            if dbg:
                nc.sync.dma_start(dbg_qi.ap().rearrange("p (c t) -> p c t", c=4), qiT[:])
                nc.sync.dma_start(dbg_ki.ap().rearrange("p (c t) -> p c t", c=4), kiT[:])

        # stats tensors for attention (small, persist to end)
        stats = stack.enter_context(tc.tile_pool(name="stats", bufs=1))
        m_all = stats.tile([P, P], F32, name="m_all")    # rowmax(S_int), col t*4+j
        z_all = stats.tile([P, P], F32, name="z_all")    # rowsum(exp(a*(S-m)))
        negm = stats.tile([P, P], F32, name="negm")      # -alpha*m
        Bini = stats.tile([P, P], F32, name="Bini")      # m + ln(Z*s_a)/alpha
        Bt = stats.tile([P, P], F32, name="Bt")          # Bini transposed

        # ============ stage 3b: v projection (AR for v overlaps pass 1) ============
        with tc.tile_pool(name="vpool", bufs=1) as vpool:
            v_sb = vpool.tile([P, 16, F], F32, name="v_sb")
            with tc.tile_pool(name="v_ps", bufs=4, space="PSUM") as v_ps:
                for i in range(16):
                    ps = v_ps.tile([P, 512], F32, name="vps")
                    for c in range(4):
                        nc.tensor.matmul(ps[:], lhsT=xT[:, c, i * P:(i + 1) * P],
                                         rhs=w_sb["wv"][:, c, :],
                                         start=(c == 0), stop=(c == 3))
                    nc.vector.scalar_tensor_tensor(v_sb[:, i, :], ps[:], scb_v[:, :1],
                                                   bv_rep[:], OP.mult, OP.add)
            vm1 = tiny.tile([P, 1], F32, name="vm1")
            nc.vector.tensor_reduce(vm1[:], v_sb[:], axis=AX.XY, op=OP.max,
                                    apply_absolute_value=True)
            vmg = cross_part_max(vm1[:], "vmg")
            ar2v = all_reduce(vmg[:1, :], 1, OP.max, "v")

            # ============ stage 5: attention pass 1 (stats) ============
            with (
                tc.tile_pool(name="s_ps", bufs=2, space="PSUM") as s_ps,
                tc.tile_pool(name="escr", bufs=2) as escr,
            ):
                for b in range(BC):
                    for h in range(H):
                        t = b * H + h
                        hp, hc = (h % 2) * 64, h // 2
                        ps = s_ps.tile([P, 4, 512], F32, name="sps")
                        for j in range(4):
                            nc.tensor.matmul(
                                ps[:, j, :],
                                lhsT=qiT[hp:hp + 64, hc,
                                         b * 512 + j * P: b * 512 + (j + 1) * P],
                                rhs=kiT[hp:hp + 64, hc, b * 512:(b + 1) * 512],
                                start=True, stop=True)
                        nc.vector.tensor_reduce(m_all[:, t * 4:(t + 1) * 4],
                                                ps[:], axis=AX.X, op=OP.max)
                        nc.vector.tensor_scalar(negm[:, t * 4:(t + 1) * 4],
                                                m_all[:, t * 4:(t + 1) * 4],
                                                nalb[:, :1], None, OP.mult)
                        for j in range(4):
                            es = escr.tile([P, 512], F32, name="es")
                            nc.scalar.activation(
                                es[:], ps[:, j, :], AF.Exp,
                                bias=negm[:, t * 4 + j:t * 4 + j + 1],
                                scale=alb[:, :1],
                                accum_out=z_all[:, t * 4 + j:t * 4 + j + 1])

                # Zmin -> AR3 -> s_a ; Bini = m + ln(Z*s_a)/alpha
                zr = tiny.tile([P, 1], F32, name="zr")
                nc.vector.tensor_reduce(zr[:], z_all[:], axis=AX.X, op=OP.min)
                nzr = tiny.tile([P, 1], F32, name="nzr")
                nc.vector.tensor_scalar(nzr[:], zr[:], -1.0, None, OP.mult)
                nzg = cross_part_max(nzr[:], "nzg")
                ar3 = all_reduce(nzg[:1, :], 1, OP.max, "zmin")
                zmin = tiny.tile([1, 1], F32, name="zmin")
                nc.vector.tensor_scalar(zmin[:], ar3[:], -1.0, None, OP.mult)
                amax = tiny.tile([1, 1], F32, name="amax")
                nc.vector.reciprocal(amax[:], zmin[:])
                s_a = tiny.tile([1, 1], F32, name="s_a")
                nc.vector.tensor_scalar(s_a[:], amax[:], 1.0 / QMAX, 1e-8,
                                        OP.mult, OP.max)
                sab = bcast(s_a[:1, :], "sab")
                zs = tiny.tile([P, P], F32, name="zs")
                nc.vector.tensor_scalar(zs[:], z_all[:], sab[:, :1], None, OP.mult)
                lnzs = tiny.tile([P, P], F32, name="lnzs")
                nc.scalar.activation(lnzs[:], zs[:], AF.Ln, bias=0.0, scale=1.0)
                nc.vector.scalar_tensor_tensor(Bini[:], lnzs[:], rab[:, :1],
                                               m_all[:], OP.mult, OP.add)
                if dbg:
                    nc.sync.dma_start(dbg_mz.ap()[:, 0:P], m_all[:])
                    nc.sync.dma_start(dbg_mz.ap()[:, P:2 * P], z_all[:])

            # fq(v) using AR2v result (overlapped with pass 1 above)
            s_v, r_v = scale_of(ar2v[:1, :], "v")
            r_vb = bcast(r_v[:1, :], "r_vb")
            with tc.tile_pool(name="scr_v", bufs=2) as scr_v:
                for c in range(4):
                    tq = scr_v.tile([P, TC], F32, name="vqnt")
                    nc.vector.tensor_scalar(tq[:], v_sb[:, 4 * c:4 * (c + 1), :],
                                            r_vb[:, :1], MAGIC_S, OP.mult, OP.add)
                    nc.vector.tensor_scalar(vi[:, 4 * c:4 * (c + 1), :], tq[:],
                                            MAGIC_S, None, OP.subtract)

        # transpose Bini -> Bt[t*4+j, p]
        with tc.tile_pool(name="bt_ps", bufs=1, space="PSUM") as bt_ps:
            bp = bt_ps.tile([P, P], F32, name="btp")
            nc.tensor.transpose(bp[:], Bini[:], ident[:])
            nc.scalar.activation(Bt[:], bp[:], AF.Identity, bias=0.0, scale=1.0)

        # ============ stage 6: attention pass 2 + AV ============
        ctxp = stack.enter_context(tc.tile_pool(name="ctxp", bufs=1))
        ctxT = ctxp.tile([P, 4, TC], F32, name="ctxT")       # ctx^T [fi, tok]
        ctxiT = ctxp.tile([P, 4, TC], BF16, name="ctxiT")
        sav = tiny.tile([1, 1], F32, name="sav")
        nc.vector.tensor_tensor(sav[:], s_a[:], s_v[:], OP.mult)
        savb = bcast(sav[:1, :], "savb")
        with (
            tc.tile_pool(name="u_ps", bufs=6, space="PSUM") as u_ps,
            tc.tile_pool(name="c_ps", bufs=2, space="PSUM") as c_ps,
            tc.tile_pool(name="tf", bufs=2) as tf_pool,
            tc.tile_pool(name="ai", bufs=2) as ai_pool,
            tc.tile_pool(name="bstg", bufs=4) as bstg_pool,
        ):
            ctx_ps = None
            for b in range(BC):
                for h in range(H):
                    t = b * H + h
                    hp, hc = (h % 2) * 64, h // 2
                    bstg = bstg_pool.tile([1, 512], F32, name="bstg")
                    nc.sync.dma_start(bstg[:], Bt[t * 4:(t + 1) * 4, :])
                    t_f32 = tf_pool.tile([P, 4, 512], F32, name="t_f32")
                    for jj in range(4):
                        up = u_ps.tile([P, 512], F32, name="up")
                        nc.tensor.matmul(
                            up[:],
                            lhsT=kiT[hp:hp + 64, hc,
                                     b * 512 + jj * P: b * 512 + (jj + 1) * P],
                            rhs=qiT[hp:hp + 64, hc, b * 512:(b + 1) * 512],
                            start=True, stop=False)
                        nc.tensor.matmul(up[:], lhsT=neg1[:], rhs=bstg[:],
                                         start=False, stop=True)
                        nc.scalar.activation(t_f32[:, jj, :], up[:], AF.Exp,
                                             bias=0.0, scale=alb[:, :1])
                    ai = ai_pool.tile([P, 4, 512], BF16, name="ai")
                    nc.vector.tensor_scalar(ai[:], t_f32[:], MAGIC_U, MAGIC_U,
                                            OP.add, OP.subtract)
                    if dbg and t == 0:
                        nc.sync.dma_start(
                            dbg_ai.ap().rearrange("p (c t2) -> p c t2", c=4), ai[:])
                    if h % 2 == 0:
                        ctx_ps = c_ps.tile([P, 512], F32, name="ctxps")
                    for jj in range(4):
                        nc.tensor.matmul(
                            ctx_ps[hp:hp + 64, :],
                            lhsT=vi[:, b * 4 + jj, h * 64:(h + 1) * 64],
                            rhs=ai[:, jj, :],
                            start=(jj == 0), stop=(jj == 3))
                    if h % 2 == 1:
                        nc.scalar.activation(ctxT[:, hc, b * 512:(b + 1) * 512],
                                             ctx_ps[:], AF.Identity,
                                             bias=0.0, scale=savb[:, :1])

            # ctx absmax -> AR4 -> fq(ctx)
            cm1 = tiny.tile([P, 1], F32, name="cm1")
            nc.vector.tensor_reduce(cm1[:], ctxT[:], axis=AX.XY, op=OP.max,
                                    apply_absolute_value=True)
            cmg = cross_part_max(cm1[:], "cmg")
            ar4 = all_reduce(cmg[:1, :], 1, OP.max, "ctx")
            s_c, r_c = scale_of(ar4[:1, :], "c")
            r_cb = bcast(r_c[:1, :], "r_cb")
            for c in range(4):
                tq = tf_pool.tile([P, TC], F32, name="ctxq")
                nc.vector.tensor_scalar(tq[:], ctxT[:, c], r_cb[:, :1], MAGIC_S,
                                        OP.mult, OP.add)
                nc.vector.tensor_scalar(ctxiT[:, c], tq[:], MAGIC_S, None,
                                        OP.subtract)
            if dbg:
                nc.sync.dma_start(dbg_ci.ap().rearrange("p (c t) -> p c t", c=4),
                                  ctxiT[:])

        # ============ stage 7: output projection + final fq ============
        sc_o = tiny.tile([1, 1], F32, name="sc_o")
        nc.vector.tensor_tensor(sc_o[:], s_c[:], wscl[:1, 3:4], OP.mult)
        scb_o = bcast(sc_o[:1, :], "scb_o")
        with (
            tc.tile_pool(name="outp", bufs=1) as outp,
            tc.tile_pool(name="o_ps", bufs=4, space="PSUM") as o_ps,
            tc.tile_pool(name="oscr", bufs=2) as oscr,
        ):
            out_sb = outp.tile([P, 16, F], F32, name="out_sb")
            for i in range(16):
                ps = o_ps.tile([P, 512], F32, name="ops")
                for c in range(4):
                    nc.tensor.matmul(ps[:], lhsT=ctxiT[:, c, i * P:(i + 1) * P],
                                     rhs=w_sb["wo"][:, c, :],
                                     start=(c == 0), stop=(c == 3))
                nc.vector.scalar_tensor_tensor(out_sb[:, i, :], ps[:], scb_o[:, :1],
                                               bo_rep[:], OP.mult, OP.add)
            om1 = tiny.tile([P, 1], F32, name="om1")
            nc.vector.tensor_reduce(om1[:], out_sb[:], axis=AX.XY, op=OP.max,
                                    apply_absolute_value=True)
            omg = cross_part_max(om1[:], "omg")
            ar5 = all_reduce(omg[:1, :], 1, OP.max, "out")
            s_o, r_o = scale_of(ar5[:1, :], "o")
            r_ob = bcast(r_o[:1, :], "r_ob")
            s_ob = bcast(s_o[:1, :], "s_ob")
            for c in range(4):
                tq = oscr.tile([P, TC], F32, name="outq")
                nc.vector.tensor_scalar(tq[:], out_sb[:, 4 * c:4 * (c + 1), :],
                                        r_ob[:, :1], MAGIC_S, OP.mult, OP.add)
                nc.vector.tensor_scalar(out_sb[:, 4 * c:4 * (c + 1), :], tq[:],
                                        MAGIC_S, s_ob[:, :1], OP.subtract, OP.mult)
            nc.sync.dma_start(out_d.ap().rearrange("(n p) f -> p n f", p=P),
                              out_sb[:])
            if dbg:
                ds = tiny.tile([1, 8], F32, name="ds")
                for i, src in enumerate((s_x[:1, :], s2t[:1, 0:1], s2t[:1, 1:2],
                                         s_v[:1, :], s_a[:1, :], s_c[:1, :],
                                         s_o[:1, :], alpha[:1, :])):
                    nc.vector.tensor_copy(ds[:1, i:i + 1], src)
                nc.sync.dma_start(dbg_scl.ap(), ds[:])

    nc.compile()
    return nc


def _fq_np(w):
    """Host-side per-tensor int8 fake-quant (matches reference fq in fp32)."""
    w = np.asarray(w, np.float32)
    s = np.maximum(np.abs(w).max() / np.float32(QMAX), np.float32(1e-8))
    wi = np.clip(np.round(w / s), -128, 127).astype(np.float32)
    return wi, np.float32(s)


def kernel(input_tensor, sequence_mask, ln_gamma, ln_beta,
           Wq, bq, Wk, bk, Wv, bv, Wo, bo):
    input_tensor = np.asarray(input_tensor, np.float32)
    assert np.asarray(sequence_mask).all(), "kernel specialized for all-true mask"
    assert np.all(np.asarray(ln_gamma) == 1.0) and np.all(np.asarray(ln_beta) == 0.0), \
        "kernel specialized for identity layernorm affine"

    if "nc" not in _CACHE:
        _CACHE["nc"] = _build()
    nc = _CACHE["nc"]

    wmaps = {}
    wscl = np.zeros((1, 4), np.float32)
    for i, (name, w) in enumerate((("wq", Wq), ("wk", Wk), ("wv", Wv), ("wo", Wo))):
        wi, s = _fq_np(w)
        wmaps[name] = np.ascontiguousarray(wi.T).astype(ml_dtypes.bfloat16)
        wscl[0, i] = s
    biases = {"bq": bq, "bk": bk, "bv": bv, "bo": bo}

    in_maps = []
    for core in range(NCORES):
        m = {"x": np.ascontiguousarray(
                 input_tensor[core * BC:(core + 1) * BC].reshape(TC, F)),
             "wscl": wscl}
        m.update(wmaps)
        for n, v in biases.items():
            m[n] = np.ascontiguousarray(np.asarray(v, np.float32))
        in_maps.append(m)

    res = run_bass_kernel_spmd(nc, in_maps, core_ids=list(range(NCORES)),
                               **_CACHE.get("run_kwargs", {}))
    _CACHE["last_result"] = res
    out = np.concatenate([r["out"].reshape(BC, T, F) for r in res.results], axis=0)
    return out


# revision 20
# speedup vs baseline: 1.0961x; 1.0961x over previous
"""Trainium2 Bass kernel for quantized Conformer MHSA (nn_ConformerMHSAWithGateV1).

Sharding: data-parallel over batch B=32 across 8 cores (4 batches/core).
All per-tensor fake-quant scales are global -> 6 tiny AllReduces (max/min),
plus one dummy warm-up collective overlapped with the input load.

Numerics strategy:
  - fq() produces integer grids |v|<=128 -> bf16 matmul operands are EXACT.
  - round() via fp32 magic-constant trick (RNE, matches jnp.round).
  - softmax: max(attn_row) == 1/Z_row exactly, so the global fq(attn) scale
    needs only AllReduce-min(Z); pass-2 recomputes scores in row layout where
    the full bias -alpha*m - ln(Z*s_a) is per-partition, the ACT exp emits
    attn/s_a directly, and the rounded bf16 ints are transposed for the AV
    matmul by the DMA xbar through a DRAM bounce (idle DMA engines).
"""
import sys

sys.path.insert(0, "/opt/trn_rl_repo")

import numpy as np
import ml_dtypes

import concourse.bass as bass
import concourse.mybir as mybir
import concourse.tile as tile
from concourse import bacc, bass_isa
from concourse.bass_utils import run_bass_kernel_spmd
from concourse.masks import make_identity

F32 = mybir.dt.float32
BF16 = mybir.dt.bfloat16
AX = mybir.AxisListType
OP = mybir.AluOpType
AF = mybir.ActivationFunctionType

B, T, F, H, DK = 32, 512, 512, 8, 64
NCORES = 8
BC = B // NCORES          # batches per core
TC = BC * T               # token rows per core
P = 128
QMAX = 127.0
LN_EPS = 1e-5
MAGIC_S = 12582912.0      # 1.5*2^23: round-to-int for signed fp32 in [-2^22, 2^22]
MAGIC_U = 8388608.0       # 2^23:     round-to-int for fp32 in [0, 2^23)
RG = [list(range(NCORES))]

_CACHE = {}


def _build(dbg=False):
    import contextlib

    nc = bacc.Bacc(None, target_bir_lowering=False, debug=False)

    x_in = nc.dram_tensor("x", [TC, F], F32, kind="ExternalInput")
    w_in = {n: nc.dram_tensor(n, [F, F], BF16, kind="ExternalInput")
            for n in ("wq", "wk", "wv", "wo")}
    b_in = {n: nc.dram_tensor(n, [F], F32, kind="ExternalInput")
            for n in ("bq", "bk", "bv", "bo")}
    wscl_in = nc.dram_tensor("wscl", [1, 4], F32, kind="ExternalInput")
    out_d = nc.dram_tensor("out", [TC, F], F32, kind="ExternalOutput")
    if dbg:
        dbg_scl = nc.dram_tensor("dbg_scl", [1, 8], F32, kind="ExternalOutput")
        dbg_xi = nc.dram_tensor("dbg_xi", [P, 4 * TC], BF16, kind="ExternalOutput")
        dbg_qi = nc.dram_tensor("dbg_qi", [P, 4 * TC], BF16, kind="ExternalOutput")
        dbg_ki = nc.dram_tensor("dbg_ki", [P, 4 * TC], BF16, kind="ExternalOutput")
        dbg_mz = nc.dram_tensor("dbg_mz", [P, 2 * P], F32, kind="ExternalOutput")
        dbg_ai = nc.dram_tensor("dbg_ai", [P, 4 * 512], BF16, kind="ExternalOutput")
        dbg_ci = nc.dram_tensor("dbg_ci", [P, 4 * TC], BF16, kind="ExternalOutput")

    with tile.TileContext(nc) as tc, contextlib.ExitStack() as stack:
        const = stack.enter_context(tc.tile_pool(name="const", bufs=1))
        tiny = stack.enter_context(tc.tile_pool(name="tiny", bufs=1))
        dram = stack.enter_context(tc.tile_pool(name="dram", bufs=1, space="DRAM"))
        persist = stack.enter_context(tc.tile_pool(name="persist", bufs=1))

        def bcast(src11, name):
            t = tiny.tile([P, 1], F32, name=name)
            nc.gpsimd.partition_broadcast(t[:], src11)
            return t

        def cross_part_max(vec, name):
            r = tiny.tile([P, 1], F32, name=name)
            nc.gpsimd.partition_all_reduce(r[:], vec, channels=P,
                                           reduce_op=bass_isa.ReduceOp.max)
            return r

        def all_reduce(src, n, op, name):
            ci = dram.tile([1, n], F32, name=f"cci_{name}")
            co = dram.tile([1, n], F32, addr_space="Shared", name=f"cco_{name}")
            nc.sync.dma_start(ci[:], src)
            nc.gpsimd.collective_compute(
                "AllReduce", op, replica_groups=RG,
                ins=[ci[:].opt()], outs=[co[:].opt()])
            r = tiny.tile([1, n], F32, name=f"ar_{name}")
            nc.sync.dma_start(r[:], co[:])
            return r

        def scale_of(armax, name):
            s = tiny.tile([1, 1], F32, name=f"s_{name}")
            nc.vector.tensor_scalar(s[:], armax, 1.0 / QMAX, 1e-8, OP.mult, OP.max)
            r = tiny.tile([1, 1], F32, name=f"r_{name}")
            nc.vector.reciprocal(r[:], s[:])
            return s, r

        # ---- constants / params ----
        ident = const.tile([P, P], F32, name="ident")
        make_identity(nc, ident)
        warm = tiny.tile([1, 1], F32, name="warm")
        nc.gpsimd.memset(warm[:], 1.0)
        all_reduce(warm[:1, :], 1, OP.max, "warmup")  # absorb first-cc overhead
        w_sb = {}
        for n in ("wq", "wk", "wv", "wo"):
            w_sb[n] = const.tile([P, 4, F], BF16, name=f"{n}_sb")
            nc.sync.dma_start(w_sb[n][:], w_in[n].ap().rearrange("(c p) f -> p c f", p=P))
        bq_sb = const.tile([P, 4], F32, name="bq_sb")
        bk_sb = const.tile([P, 4], F32, name="bk_sb")
        nc.sync.dma_start(bq_sb[:], b_in["bq"].ap().rearrange("(a p) -> p a", p=P))
        nc.sync.dma_start(bk_sb[:], b_in["bk"].ap().rearrange("(a p) -> p a", p=P))
        bv_rep = const.tile([P, F], F32, name="bv_rep")
        bo_rep = const.tile([P, F], F32, name="bo_rep")
        nc.sync.dma_start(bv_rep[:1, :], b_in["bv"].ap().rearrange("(o f) -> o f", o=1))
        nc.gpsimd.partition_broadcast(bv_rep[:], bv_rep[:1, :])
        nc.sync.dma_start(bo_rep[:1, :], b_in["bo"].ap().rearrange("(o f) -> o f", o=1))
        nc.gpsimd.partition_broadcast(bo_rep[:], bo_rep[:1, :])
        wscl = tiny.tile([1, 4], F32, name="wscl")
        nc.sync.dma_start(wscl[:], wscl_in.ap())

        # ---- persistent int activations ----
        xT = persist.tile([P, 4, TC], BF16, name="xT")       # x_int^T [fi, tok]
        qiT = persist.tile([P, 4, TC], BF16, name="qiT")     # q_int^T [fo, tok]
        kiT = persist.tile([P, 4, TC], BF16, name="kiT")
        vi = persist.tile([P, 16, F], BF16, name="vi")       # v_int   [tok, fo]

        # ============ stages 1-2: layernorm, transpose, AR1, fq(x) ============
        with (
            tc.tile_pool(name="early_a", bufs=1) as early_a,
            tc.tile_pool(name="scr_a", bufs=2) as scr_a,
            tc.tile_pool(name="tp_ps", bufs=4, space="PSUM") as tp_ps,
        ):
            x_sb = early_a.tile([P, 16, F], F32, name="x_sb")
            nc.sync.dma_start(x_sb[:], x_in.ap().rearrange("(n p) f -> p n f", p=P))
            mu = early_a.tile([P, 16], F32, name="mu")
            ssq = early_a.tile([P, 16], F32, name="ssq")
            rmaxt = early_a.tile([P, 16], F32, name="rmaxt")
            for i in range(16):
                s1 = scr_a.tile([P, F], F32, name="lnscr")
                nc.scalar.activation(s1[:], x_sb[:, i], AF.Copy, bias=0.0,
                                     scale=1.0 / F, accum_out=mu[:, i:i + 1])
                nc.vector.tensor_scalar(x_sb[:, i], x_sb[:, i], mu[:, i:i + 1],
                                        None, OP.subtract)
                s2 = scr_a.tile([P, F], F32, name="lnscr")
                nc.scalar.activation(s2[:], x_sb[:, i], AF.Square, bias=0.0,
                                     scale=1.0, accum_out=ssq[:, i:i + 1])
                nc.vector.tensor_reduce(rmaxt[:, i:i + 1], x_sb[:, i], axis=AX.X,
                                        op=OP.max, apply_absolute_value=True)
            var = early_a.tile([P, 16], F32, name="var")
            nc.vector.tensor_scalar(var[:], ssq[:], 1.0 / F, LN_EPS, OP.mult, OP.add)
            stdv = early_a.tile([P, 16], F32, name="stdv")
            nc.scalar.activation(stdv[:], var[:], AF.Sqrt, bias=0.0, scale=1.0)
            rstd = early_a.tile([P, 16], F32, name="rstd")
            nc.vector.reciprocal(rstd[:], stdv[:])
            # absmax of LN output = rstd * rowmax|x-mu| (monotone => exact);
            # issue AR1 early, then transpose x_n = t*rstd while it flies.
            gx = early_a.tile([P, 16], F32, name="gx")
            nc.vector.tensor_tensor(gx[:], rstd[:], rmaxt[:], OP.mult)
            gx1 = early_a.tile([P, 1], F32, name="gx1")
            nc.vector.tensor_reduce(gx1[:], gx[:], axis=AX.X, op=OP.max)
            gxm = cross_part_max(gx1[:], "gxm")
            ar1 = all_reduce(gxm[:1, :], 1, OP.max, "sx")

            xnT = early_a.tile([P, 4, TC], F32, name="xnT")
            for i in range(16):
                u = scr_a.tile([P, F], F32, name="uq")
                nc.vector.tensor_scalar(u[:], x_sb[:, i], rstd[:, i:i + 1],
                                        None, OP.mult)
                for j in range(4):
                    pt = tp_ps.tile([P, P], F32, name="tpp")
                    nc.tensor.transpose(pt[:], u[:, j * P:(j + 1) * P], ident[:])
                    nc.scalar.activation(xnT[:, j, i * P:(i + 1) * P], pt[:],
                                         AF.Identity, bias=0.0, scale=1.0)
            s_x, r_x = scale_of(ar1[:1, :], "x")
            r_xb = bcast(r_x[:1, :], "r_xb")
            for c in range(4):
                tq = scr_a.tile([P, TC], F32, name="xqnt")
                nc.vector.tensor_scalar(tq[:], xnT[:, c], r_xb[:, :1], MAGIC_S,
                                        OP.mult, OP.add)
                nc.vector.tensor_scalar(xT[:, c], tq[:], MAGIC_S, None, OP.subtract)
            if dbg:
                nc.sync.dma_start(dbg_xi.ap().rearrange("p (c t) -> p c t", c=4), xT[:])

        # ============ stage 3: q,k,v projections; AR2qk overlaps v ============
        sc_q = tiny.tile([1, 1], F32, name="sc_q")
        sc_k = tiny.tile([1, 1], F32, name="sc_k")
        sc_v = tiny.tile([1, 1], F32, name="sc_v")
        nc.vector.tensor_tensor(sc_q[:], s_x[:], wscl[:1, 0:1], OP.mult)
        nc.vector.tensor_tensor(sc_k[:], s_x[:], wscl[:1, 1:2], OP.mult)
        nc.vector.tensor_tensor(sc_v[:], s_x[:], wscl[:1, 2:3], OP.mult)
        scb_q = bcast(sc_q[:1, :], "scb_q")
        scb_k = bcast(sc_k[:1, :], "scb_k")
        scb_v = bcast(sc_v[:1, :], "scb_v")

        stats = stack.enter_context(tc.tile_pool(name="stats", bufs=1))
        m_all = stats.tile([P, P], F32, name="m_all")    # rowmax(S_int), col t*4+j
        z_all = stats.tile([P, P], F32, name="z_all")    # rowsum(exp(a*(S-m)))
        negm = stats.tile([P, P], F32, name="negm")      # -alpha*m
        bias2 = stats.tile([P, P], F32, name="bias2")    # -alpha*m - ln(Z*s_a)

        with (
            tc.tile_pool(name="early_b", bufs=1) as early_b,
            tc.tile_pool(name="scr_b", bufs=2) as scr_b,
            tc.tile_pool(name="mm_ps", bufs=4, space="PSUM") as mm_ps,
        ):
            qT = early_b.tile([P, 4, TC], F32, name="qT")
            kT = early_b.tile([P, 4, TC], F32, name="kT")
            v_sb = early_b.tile([P, 16, F], F32, name="v_sb")
            for wname, dstT, bsb, scb in (("wq", qT, bq_sb, scb_q),
                                          ("wk", kT, bk_sb, scb_k)):
                for a in range(4):
                    for n in range(4):
                        ps = mm_ps.tile([P, 512], F32, name="qkps")
                        for c in range(4):
                            nc.tensor.matmul(
                                ps[:], lhsT=w_sb[wname][:, c, a * P:(a + 1) * P],
                                rhs=xT[:, c, n * 512:(n + 1) * 512],
                                start=(c == 0), stop=(c == 3))
                        nc.scalar.activation(dstT[:, a, n * 512:(n + 1) * 512],
                                             ps[:], AF.Identity,
                                             bias=bsb[:, a:a + 1], scale=scb[:, :1])
            vals2 = tiny.tile([1, 2], F32, name="vals2")
            for idx, src in enumerate((qT, kT)):
                r1 = tiny.tile([P, 1], F32, name=f"qkmax{idx}")
                nc.vector.tensor_reduce(r1[:], src[:], axis=AX.XY, op=OP.max,
                                        apply_absolute_value=True)
                rm = cross_part_max(r1[:], f"qkgm{idx}")
                nc.vector.tensor_copy(vals2[:1, idx:idx + 1], rm[:1, :])
            ar2 = all_reduce(vals2[:1, :], 2, OP.max, "qk")

            # v projection (overlaps AR2qk)
            for i in range(16):
                ps = mm_ps.tile([P, 512], F32, name="qkps")
                for c in range(4):
                    nc.tensor.matmul(ps[:], lhsT=xT[:, c, i * P:(i + 1) * P],
                                     rhs=w_sb["wv"][:, c, :],
                                     start=(c == 0), stop=(c == 3))
                nc.vector.scalar_tensor_tensor(v_sb[:, i, :], ps[:], scb_v[:, :1],
                                               bv_rep[:], OP.mult, OP.add)
            vm1 = tiny.tile([P, 1], F32, name="vm1")
            nc.vector.tensor_reduce(vm1[:], v_sb[:], axis=AX.XY, op=OP.max,
                                    apply_absolute_value=True)
            vmg = cross_part_max(vm1[:], "vmg")
            ar2v = all_reduce(vmg[:1, :], 1, OP.max, "v")

            s2t = tiny.tile([1, 2], F32, name="s2t")
            nc.vector.tensor_scalar(s2t[:], ar2[:], 1.0 / QMAX, 1e-8, OP.mult, OP.max)
            r2t = tiny.tile([1, 2], F32, name="r2t")
            nc.vector.reciprocal(r2t[:], s2t[:])
            r_qb = bcast(r2t[:1, 0:1], "r_qb")
            r_kb = bcast(r2t[:1, 1:2], "r_kb")
            # alpha = s_q*s_k/8  (scores scale; /sqrt(64) == /8 exact)
            alpha = tiny.tile([1, 1], F32, name="alpha")
            nc.vector.tensor_tensor(alpha[:], s2t[:1, 0:1], s2t[:1, 1:2], OP.mult)
            nc.vector.tensor_scalar(alpha[:], alpha[:], 0.125, None, OP.mult)
            nalpha = tiny.tile([1, 1], F32, name="nalpha")
            nc.vector.tensor_scalar(nalpha[:], alpha[:], -1.0, None, OP.mult)
            alb = bcast(alpha[:1, :], "alb")
            nalb = bcast(nalpha[:1, :], "nalb")

            for src, dst, rb in ((qT, qiT, r_qb), (kT, kiT, r_kb)):
                for c in range(4):
                    tq = scr_b.tile([P, TC], F32, name="tqnt")
                    nc.vector.tensor_scalar(tq[:], src[:, c], rb[:, :1], MAGIC_S,
                                            OP.mult, OP.add)
                    nc.vector.tensor_scalar(dst[:, c], tq[:], MAGIC_S, None,
                                            OP.subtract)
            if dbg:
                nc.sync.dma_start(dbg_qi.ap().rearrange("p (c t) -> p c t", c=4), qiT[:])
                nc.sync.dma_start(dbg_ki.ap().rearrange("p (c t) -> p c t", c=4), kiT[:])

            # fq(v) with AR2v result
            s_v, r_v = scale_of(ar2v[:1, :], "v")
            r_vb = bcast(r_v[:1, :], "r_vb")
            for c in range(4):
                tq = scr_b.tile([P, TC], F32, name="tqnt")
                nc.vector.tensor_scalar(tq[:], v_sb[:, 4 * c:4 * (c + 1), :],
                                        r_vb[:, :1], MAGIC_S, OP.mult, OP.add)
                nc.vector.tensor_scalar(vi[:, 4 * c:4 * (c + 1), :], tq[:],
                                        MAGIC_S, None, OP.subtract)

        # ============ stage 5: attention pass 1 (stats) ============
        with (
            tc.tile_pool(name="s_ps", bufs=2, space="PSUM") as s_ps,
            tc.tile_pool(name="escr", bufs=2) as escr,
        ):
            for b in range(BC):
                for h in range(H):
                    t = b * H + h
                    hp, hc = (h % 2) * 64, h // 2
                    ps = s_ps.tile([P, 4, 512], F32, name="sps")
                    for j in range(4):
                        nc.tensor.matmul(
                            ps[:, j, :],
                            lhsT=qiT[hp:hp + 64, hc,
                                     b * 512 + j * P: b * 512 + (j + 1) * P],
                            rhs=kiT[hp:hp + 64, hc, b * 512:(b + 1) * 512],
                            start=True, stop=True)
                    nc.vector.tensor_reduce(m_all[:, t * 4:(t + 1) * 4],
                                            ps[:], axis=AX.X, op=OP.max)
                    nc.vector.tensor_scalar(negm[:, t * 4:(t + 1) * 4],
                                            m_all[:, t * 4:(t + 1) * 4],
                                            nalb[:, :1], None, OP.mult)
                    for j in range(4):
                        es = escr.tile([P, 512], F32, name="es")
                        nc.scalar.activation(
                            es[:], ps[:, j, :], AF.Exp,
                            bias=negm[:, t * 4 + j:t * 4 + j + 1],
                            scale=alb[:, :1],
                            accum_out=z_all[:, t * 4 + j:t * 4 + j + 1])

            # Zmin -> AR3 -> s_a ; bias2 = -alpha*m - ln(Z*s_a)
            zr = tiny.tile([P, 1], F32, name="zr")
            nc.vector.tensor_reduce(zr[:], z_all[:], axis=AX.X, op=OP.min)
            nzr = tiny.tile([P, 1], F32, name="nzr")
            nc.vector.tensor_scalar(nzr[:], zr[:], -1.0, None, OP.mult)
            nzg = cross_part_max(nzr[:], "nzg")
            ar3 = all_reduce(nzg[:1, :], 1, OP.max, "zmin")
            zmin = tiny.tile([1, 1], F32, name="zmin")
            nc.vector.tensor_scalar(zmin[:], ar3[:], -1.0, None, OP.mult)
            amax = tiny.tile([1, 1], F32, name="amax")
            nc.vector.reciprocal(amax[:], zmin[:])
            s_a = tiny.tile([1, 1], F32, name="s_a")
            nc.vector.tensor_scalar(s_a[:], amax[:], 1.0 / QMAX, 1e-8,
                                    OP.mult, OP.max)
            sab = bcast(s_a[:1, :], "sab")
            zs = tiny.tile([P, P], F32, name="zs")
            nc.vector.tensor_scalar(zs[:], z_all[:], sab[:, :1], None, OP.mult)
            lnzs = tiny.tile([P, P], F32, name="lnzs")
            nc.scalar.activation(lnzs[:], zs[:], AF.Ln, bias=0.0, scale=1.0)
            nc.vector.tensor_tensor(bias2[:], negm[:], lnzs[:], OP.subtract)
            if dbg:
                nc.sync.dma_start(dbg_mz.ap()[:, 0:P], m_all[:])
                nc.sync.dma_start(dbg_mz.ap()[:, P:2 * P], z_all[:])

        # ============ stage 6: pass 2 (row layout) + DMA transpose + AV ============
        ctxp = stack.enter_context(tc.tile_pool(name="ctxp", bufs=1))
        ctxT = ctxp.tile([P, 4, TC], F32, name="ctxT")       # ctx^T [fi, tok]
        ctxiT = ctxp.tile([P, 4, TC], BF16, name="ctxiT")
        sav = tiny.tile([1, 1], F32, name="sav")
        nc.vector.tensor_tensor(sav[:], s_a[:], s_v[:], OP.mult)
        savb = bcast(sav[:1, :], "savb")
        with (
            tc.tile_pool(name="u_ps", bufs=3, space="PSUM") as u_ps,
            tc.tile_pool(name="c_ps", bufs=2, space="PSUM") as c_ps,
            tc.tile_pool(name="tf", bufs=2) as tf_pool,
            tc.tile_pool(name="air", bufs=2) as air_pool,
            tc.tile_pool(name="ait", bufs=2) as ait_pool,
            tc.tile_pool(name="aid", bufs=4, space="DRAM") as aid_pool,
        ):
            ctx_ps = None
            for b in range(BC):
                for h in range(H):
                    t = b * H + h
                    hp, hc = (h % 2) * 64, h // 2
                    ps2 = []
                    for half in range(2):
                        ph = u_ps.tile([P, 2, 512], F32, name="sps2")
                        ps2.append(ph)
                        for j2 in range(2):
                            j = half * 2 + j2
                            nc.tensor.matmul(
                                ph[:, j2, :],
                                lhsT=qiT[hp:hp + 64, hc,
                                         b * 512 + j * P: b * 512 + (j + 1) * P],
                                rhs=kiT[hp:hp + 64, hc, b * 512:(b + 1) * 512],
                                start=True, stop=True)
                    t_f32 = tf_pool.tile([P, 4, 512], F32, name="t_f32")
                    for j in range(4):
                        nc.scalar.activation(
                            t_f32[:, j, :], ps2[j // 2][:, j % 2, :], AF.Exp,
                            bias=bias2[:, t * 4 + j:t * 4 + j + 1],
                            scale=alb[:, :1])
                    ai_row = air_pool.tile([P, 4, 512], BF16, name="ai_row")
                    nc.vector.tensor_scalar(ai_row[:], t_f32[:], MAGIC_U, MAGIC_U,
                                            OP.add, OP.subtract)
                    if dbg and t == 0:
                        nc.sync.dma_start(
                            dbg_ai.ap().rearrange("p (c t2) -> p c t2", c=4),
                            ai_row[:])
                    # transpose attn ints via DMA xbar through DRAM
                    ad = aid_pool.tile([512, 512], BF16, name="aid")
                    nc.sync.dma_start(ad[:, :].rearrange("(j p) k -> p j k", p=P),
                                      ai_row[:])
                    ai_T = ait_pool.tile([P, 4, 512], BF16, name="ai_T")
                    for jj in range(4):
                        nc.sync.dma_start_transpose(
                            ai_T[:, jj, :], ad[:, jj * P:(jj + 1) * P])
                    if h % 2 == 0:
                        ctx_ps = c_ps.tile([P, 512], F32, name="ctxps")
                    for jj in range(4):
                        nc.tensor.matmul(
                            ctx_ps[hp:hp + 64, :],
                            lhsT=vi[:, b * 4 + jj, h * 64:(h + 1) * 64],
                            rhs=ai_T[:, jj, :],
                            start=(jj == 0), stop=(jj == 3))
                    if h % 2 == 1:
                        nc.scalar.activation(ctxT[:, hc, b * 512:(b + 1) * 512],
                                             ctx_ps[:], AF.Identity,
                                             bias=0.0, scale=savb[:, :1])

            # ctx absmax -> AR4 -> fq(ctx)
            cm1 = tiny.tile([P, 1], F32, name="cm1")
            nc.vector.tensor_reduce(cm1[:], ctxT[:], axis=AX.XY, op=OP.max,
                                    apply_absolute_value=True)
            cmg = cross_part_max(cm1[:], "cmg")
            ar4 = all_reduce(cmg[:1, :], 1, OP.max, "ctx")
            s_c, r_c = scale_of(ar4[:1, :], "c")
            r_cb = bcast(r_c[:1, :], "r_cb")
            for c in range(4):
                tq = tf_pool.tile([P, TC], F32, name="ctxq")
                nc.vector.tensor_scalar(tq[:], ctxT[:, c], r_cb[:, :1], MAGIC_S,
                                        OP.mult, OP.add)
                nc.vector.tensor_scalar(ctxiT[:, c], tq[:], MAGIC_S, None,
                                        OP.subtract)
            if dbg:
                nc.sync.dma_start(dbg_ci.ap().rearrange("p (c t) -> p c t", c=4),
                                  ctxiT[:])

        # ============ stage 7: output projection + final fq ============
        sc_o = tiny.tile([1, 1], F32, name="sc_o")
        nc.vector.tensor_tensor(sc_o[:], s_c[:], wscl[:1, 3:4], OP.mult)
        scb_o = bcast(sc_o[:1, :], "scb_o")
        with (
            tc.tile_pool(name="outp", bufs=1) as outp,
            tc.tile_pool(name="o_ps", bufs=4, space="PSUM") as o_ps,
            tc.tile_pool(name="oscr", bufs=2) as oscr,
        ):
            out_sb = outp.tile([P, 16, F], F32, name="out_sb")
            for i in range(16):
                ps = o_ps.tile([P, 512], F32, name="ops")
                for c in range(4):
                    nc.tensor.matmul(ps[:], lhsT=ctxiT[:, c, i * P:(i + 1) * P],
                                     rhs=w_sb["wo"][:, c, :],
                                     start=(c == 0), stop=(c == 3))
                nc.vector.scalar_tensor_tensor(out_sb[:, i, :], ps[:], scb_o[:, :1],
                                               bo_rep[:], OP.mult, OP.add)
            om1 = tiny.tile([P, 1], F32, name="om1")
            nc.vector.tensor_reduce(om1[:], out_sb[:], axis=AX.XY, op=OP.max,
                                    apply_absolute_value=True)
            omg = cross_part_max(om1[:], "omg")
            ar5 = all_reduce(omg[:1, :], 1, OP.max, "out")
            s_o, r_o = scale_of(ar5[:1, :], "o")
            r_ob = bcast(r_o[:1, :], "r_ob")
            s_ob = bcast(s_o[:1, :], "s_ob")
            for c in range(4):
                tq = oscr.tile([P, TC], F32, name="outq")
                nc.vector.tensor_scalar(tq[:], out_sb[:, 4 * c:4 * (c + 1), :],
                                        r_ob[:, :1], MAGIC_S, OP.mult, OP.add)
                nc.vector.tensor_scalar(out_sb[:, 4 * c:4 * (c + 1), :], tq[:],
                                        MAGIC_S, s_ob[:, :1], OP.subtract, OP.mult)
            nc.sync.dma_start(out_d.ap().rearrange("(n p) f -> p n f", p=P),
                              out_sb[:])
            if dbg:
                ds = tiny.tile([1, 8], F32, name="ds")
                for i, src in enumerate((s_x[:1, :], s2t[:1, 0:1], s2t[:1, 1:2],
                                         s_v[:1, :], s_a[:1, :], s_c[:1, :],
                                         s_o[:1, :], alpha[:1, :])):
                    nc.vector.tensor_copy(ds[:1, i:i + 1], src)
                nc.sync.dma_start(dbg_scl.ap(), ds[:])

    nc.compile()
    return nc


def _fq_np(w):
    """Host-side per-tensor int8 fake-quant (matches reference fq in fp32)."""
    w = np.asarray(w, np.float32)
    s = np.maximum(np.abs(w).max() / np.float32(QMAX), np.float32(1e-8))
    wi = np.clip(np.round(w / s), -128, 127).astype(np.float32)
    return wi, np.float32(s)


def kernel(input_tensor, sequence_mask, ln_gamma, ln_beta,
           Wq, bq, Wk, bk, Wv, bv, Wo, bo):
    input_tensor = np.asarray(input_tensor, np.float32)
    assert np.asarray(sequence_mask).all(), "kernel specialized for all-true mask"
    assert np.all(np.asarray(ln_gamma) == 1.0) and np.all(np.asarray(ln_beta) == 0.0), \
        "kernel specialized for identity layernorm affine"

    if "nc" not in _CACHE:
        _CACHE["nc"] = _build()
    nc = _CACHE["nc"]

    wmaps = {}
    wscl = np.zeros((1, 4), np.float32)
    for i, (name, w) in enumerate((("wq", Wq), ("wk", Wk), ("wv", Wv), ("wo", Wo))):
        wi, s = _fq_np(w)
        wmaps[name] = np.ascontiguousarray(wi.T).astype(ml_dtypes.bfloat16)
        wscl[0, i] = s
    biases = {"bq": bq, "bk": bk, "bv": bv, "bo": bo}

    in_maps = []
    for core in range(NCORES):
        m = {"x": np.ascontiguousarray(
                 input_tensor[core * BC:(core + 1) * BC].reshape(TC, F)),
             "wscl": wscl}
        m.update(wmaps)
        for n, v in biases.items():
            m[n] = np.ascontiguousarray(np.asarray(v, np.float32))
        in_maps.append(m)

    res = run_bass_kernel_spmd(nc, in_maps, core_ids=list(range(NCORES)),
                               **_CACHE.get("run_kwargs", {}))
    _CACHE["last_result"] = res
    out = np.concatenate([r["out"].reshape(BC, T, F) for r in res.results], axis=0)
    return out


# revision 23
# speedup vs baseline: 1.2388x; 1.1302x over previous
"""Trainium2 Bass kernel for quantized Conformer MHSA (nn_ConformerMHSAWithGateV1).

Sharding: data-parallel over batch B=32 across 8 cores (4 batches/core).
All per-tensor fake-quant scales are global -> 6 tiny AllReduces (max/min),
plus one dummy warm-up collective overlapped with the input load.

Numerics strategy:
  - fq() produces integer grids |v|<=128 -> bf16 matmul operands are EXACT.
  - round() via fp32 magic-constant trick (RNE, matches jnp.round).
  - softmax: max(attn_row) == 1/Z_row exactly, so the global fq(attn) scale
    needs only AllReduce-min(Z); pass-2 recomputes scores in row layout where
    the full bias -alpha*m - ln(Z*s_a) is per-partition, the ACT exp emits
    attn/s_a directly, and the rounded bf16 ints are transposed for the AV
    matmul by the DMA xbar through a DRAM bounce (idle DMA engines).
"""
import sys

sys.path.insert(0, "/opt/trn_rl_repo")

import numpy as np
import ml_dtypes

import concourse.bass as bass
import concourse.mybir as mybir
import concourse.tile as tile
from concourse import bacc, bass_isa
from concourse.bass_utils import run_bass_kernel_spmd
from concourse.masks import make_identity

F32 = mybir.dt.float32
BF16 = mybir.dt.bfloat16
FP16 = mybir.dt.float16
AX = mybir.AxisListType
OP = mybir.AluOpType
AF = mybir.ActivationFunctionType

B, T, F, H, DK = 32, 512, 512, 8, 64
NCORES = 8
BC = B // NCORES          # batches per core
TC = BC * T               # token rows per core
P = 128
QMAX = 127.0
LN_EPS = 1e-5
MAGIC_S = 12582912.0      # 1.5*2^23: round-to-int for signed fp32 in [-2^22, 2^22]
MAGIC_U = 8388608.0       # 2^23:     round-to-int for fp32 in [0, 2^23)
RG = [list(range(NCORES))]

_CACHE = {}


def _build(dbg=False):
    import contextlib

    nc = bacc.Bacc(None, target_bir_lowering=False, debug=False)

    x_in = nc.dram_tensor("x", [TC, F], F32, kind="ExternalInput")
    w_in = {n: nc.dram_tensor(n, [F, F], FP16, kind="ExternalInput")
            for n in ("wq", "wk", "wv", "wo")}
    b_in = {n: nc.dram_tensor(n, [F], F32, kind="ExternalInput")
            for n in ("bq", "bk", "bv", "bo")}
    wscl_in = nc.dram_tensor("wscl", [1, 4], F32, kind="ExternalInput")
    out_d = nc.dram_tensor("out", [TC, F], F32, kind="ExternalOutput")
    if dbg:
        dbg_scl = nc.dram_tensor("dbg_scl", [1, 8], F32, kind="ExternalOutput")
        dbg_xi = nc.dram_tensor("dbg_xi", [P, 4 * TC], FP16, kind="ExternalOutput")
        dbg_qi = nc.dram_tensor("dbg_qi", [P, 4 * TC], FP16, kind="ExternalOutput")
        dbg_ki = nc.dram_tensor("dbg_ki", [P, 4 * TC], FP16, kind="ExternalOutput")
        dbg_mz = nc.dram_tensor("dbg_mz", [P, 2 * P], F32, kind="ExternalOutput")
        dbg_ai = nc.dram_tensor("dbg_ai", [P, 4 * 512], FP16, kind="ExternalOutput")
        dbg_ci = nc.dram_tensor("dbg_ci", [P, 4 * TC], FP16, kind="ExternalOutput")

    with tile.TileContext(nc) as tc, contextlib.ExitStack() as stack:
        const = stack.enter_context(tc.tile_pool(name="const", bufs=1))
        tiny = stack.enter_context(tc.tile_pool(name="tiny", bufs=1))
        dram = stack.enter_context(tc.tile_pool(name="dram", bufs=1, space="DRAM"))
        persist = stack.enter_context(tc.tile_pool(name="persist", bufs=1))

        def bcast(src11, name):
            t = tiny.tile([P, 1], F32, name=name)
            nc.gpsimd.partition_broadcast(t[:], src11)
            return t

        def cross_part_max(vec, name):
            r = tiny.tile([P, 1], F32, name=name)
            nc.gpsimd.partition_all_reduce(r[:], vec, channels=P,
                                           reduce_op=bass_isa.ReduceOp.max)
            return r

        def all_reduce(src, n, op, name):
            ci = dram.tile([1, n], F32, name=f"cci_{name}")
            co = dram.tile([1, n], F32, addr_space="Shared", name=f"cco_{name}")
            nc.sync.dma_start(ci[:], src)
            nc.gpsimd.collective_compute(
                "AllReduce", op, replica_groups=RG,
                ins=[ci[:].opt()], outs=[co[:].opt()])
            r = tiny.tile([1, n], F32, name=f"ar_{name}")
            nc.sync.dma_start(r[:], co[:])
            return r

        def scale_of(armax, name):
            s = tiny.tile([1, 1], F32, name=f"s_{name}")
            nc.vector.tensor_scalar(s[:], armax, 1.0 / QMAX, 1e-8, OP.mult, OP.max)
            r = tiny.tile([1, 1], F32, name=f"r_{name}")
            nc.vector.reciprocal(r[:], s[:])
            return s, r

        # ---- constants / params ----
        ident = const.tile([P, P], F32, name="ident")
        make_identity(nc, ident)
        warm = tiny.tile([1, 1], F32, name="warm")
        nc.gpsimd.memset(warm[:], 1.0)
        all_reduce(warm[:1, :], 1, OP.max, "warmup")  # absorb first-cc overhead
        w_sb = {}
        for n in ("wq", "wk", "wv", "wo"):
            w_sb[n] = const.tile([P, 4, F], FP16, name=f"{n}_sb")
            nc.sync.dma_start(w_sb[n][:], w_in[n].ap().rearrange("(c p) f -> p c f", p=P))
        bq_sb = const.tile([P, 4], F32, name="bq_sb")
        bk_sb = const.tile([P, 4], F32, name="bk_sb")
        nc.sync.dma_start(bq_sb[:], b_in["bq"].ap().rearrange("(a p) -> p a", p=P))
        nc.sync.dma_start(bk_sb[:], b_in["bk"].ap().rearrange("(a p) -> p a", p=P))
        bv_rep = const.tile([P, F], F32, name="bv_rep")
        bo_rep = const.tile([P, F], F32, name="bo_rep")
        nc.sync.dma_start(bv_rep[:1, :], b_in["bv"].ap().rearrange("(o f) -> o f", o=1))
        nc.gpsimd.partition_broadcast(bv_rep[:], bv_rep[:1, :])
        nc.sync.dma_start(bo_rep[:1, :], b_in["bo"].ap().rearrange("(o f) -> o f", o=1))
        nc.gpsimd.partition_broadcast(bo_rep[:], bo_rep[:1, :])
        wscl = tiny.tile([1, 4], F32, name="wscl")
        nc.sync.dma_start(wscl[:], wscl_in.ap())

        # ---- persistent int activations ----
        xT = persist.tile([P, 4, TC], FP16, name="xT")       # x_int^T [fi, tok]
        qiT = persist.tile([P, 4, TC], FP16, name="qiT")     # q_int^T [fo, tok]
        kiT = persist.tile([P, 4, TC], FP16, name="kiT")
        vi = persist.tile([P, 16, F], FP16, name="vi")       # v_int   [tok, fo]

        # ============ stages 1-2: layernorm, transpose, AR1, fq(x) ============
        with (
            tc.tile_pool(name="early_a", bufs=1) as early_a,
            tc.tile_pool(name="scr_a", bufs=2) as scr_a,
            tc.tile_pool(name="tp_ps", bufs=4, space="PSUM") as tp_ps,
        ):
            x_sb = early_a.tile([P, 16, F], F32, name="x_sb")
            nc.sync.dma_start(x_sb[:], x_in.ap().rearrange("(n p) f -> p n f", p=P))
            mu = early_a.tile([P, 16], F32, name="mu")
            ssq = early_a.tile([P, 16], F32, name="ssq")
            rmaxt = early_a.tile([P, 16], F32, name="rmaxt")
            for i in range(16):
                s1 = scr_a.tile([P, F], F32, name="lnscr")
                nc.scalar.activation(s1[:], x_sb[:, i], AF.Copy, bias=0.0,
                                     scale=1.0 / F, accum_out=mu[:, i:i + 1])
                nc.vector.tensor_scalar(x_sb[:, i], x_sb[:, i], mu[:, i:i + 1],
                                        None, OP.subtract)
                s2 = scr_a.tile([P, F], F32, name="lnscr")
                nc.scalar.activation(s2[:], x_sb[:, i], AF.Square, bias=0.0,
                                     scale=1.0, accum_out=ssq[:, i:i + 1])
                nc.vector.tensor_reduce(rmaxt[:, i:i + 1], x_sb[:, i], axis=AX.X,
                                        op=OP.max, apply_absolute_value=True)
            var = early_a.tile([P, 16], F32, name="var")
            nc.vector.tensor_scalar(var[:], ssq[:], 1.0 / F, LN_EPS, OP.mult, OP.add)
            stdv = early_a.tile([P, 16], F32, name="stdv")
            nc.scalar.activation(stdv[:], var[:], AF.Sqrt, bias=0.0, scale=1.0)
            rstd = early_a.tile([P, 16], F32, name="rstd")
            nc.vector.reciprocal(rstd[:], stdv[:])
            # absmax of LN output = rstd * rowmax|x-mu| (monotone => exact);
            # issue AR1 early, then transpose x_n = t*rstd while it flies.
            gx = early_a.tile([P, 16], F32, name="gx")
            nc.vector.tensor_tensor(gx[:], rstd[:], rmaxt[:], OP.mult)
            gx1 = early_a.tile([P, 1], F32, name="gx1")
            nc.vector.tensor_reduce(gx1[:], gx[:], axis=AX.X, op=OP.max)
            gxm = cross_part_max(gx1[:], "gxm")
            ar1 = all_reduce(gxm[:1, :], 1, OP.max, "sx")

            xnT = early_a.tile([P, 4, TC], F32, name="xnT")
            for i in range(16):
                u = scr_a.tile([P, F], F32, name="uq")
                nc.vector.tensor_scalar(u[:], x_sb[:, i], rstd[:, i:i + 1],
                                        None, OP.mult)
                for j in range(4):
                    pt = tp_ps.tile([P, P], F32, name="tpp")
                    nc.tensor.transpose(pt[:], u[:, j * P:(j + 1) * P], ident[:])
                    nc.scalar.activation(xnT[:, j, i * P:(i + 1) * P], pt[:],
                                         AF.Identity, bias=0.0, scale=1.0)
            s_x, r_x = scale_of(ar1[:1, :], "x")
            r_xb = bcast(r_x[:1, :], "r_xb")
            for c in range(4):
                tq = scr_a.tile([P, TC], F32, name="xqnt")
                nc.vector.tensor_scalar(tq[:], xnT[:, c], r_xb[:, :1], MAGIC_S,
                                        OP.mult, OP.add)
                nc.vector.tensor_scalar(xT[:, c], tq[:], MAGIC_S, None, OP.subtract)
            if dbg:
                nc.sync.dma_start(dbg_xi.ap().rearrange("p (c t) -> p c t", c=4), xT[:])

        # ============ stage 3: q,k,v projections; AR2qk overlaps v ============
        sc_q = tiny.tile([1, 1], F32, name="sc_q")
        sc_k = tiny.tile([1, 1], F32, name="sc_k")
        sc_v = tiny.tile([1, 1], F32, name="sc_v")
        nc.vector.tensor_tensor(sc_q[:], s_x[:], wscl[:1, 0:1], OP.mult)
        nc.vector.tensor_tensor(sc_k[:], s_x[:], wscl[:1, 1:2], OP.mult)
        nc.vector.tensor_tensor(sc_v[:], s_x[:], wscl[:1, 2:3], OP.mult)
        scb_q = bcast(sc_q[:1, :], "scb_q")
        scb_k = bcast(sc_k[:1, :], "scb_k")
        scb_v = bcast(sc_v[:1, :], "scb_v")

        stats = stack.enter_context(tc.tile_pool(name="stats", bufs=1))
        m_all = stats.tile([P, P], F32, name="m_all")    # rowmax(S_int), col t*4+j
        z_all = stats.tile([P, P], F32, name="z_all")    # rowsum(exp(a*(S-m)))
        negm = stats.tile([P, P], F32, name="negm")      # -alpha*m
        mw = stats.tile([P, P], F32, name="mw")          # m + ln(Z*s_a)/alpha
        digT = stats.tile([P, 3, P], FP16, name="digT")  # fp16 cascade of mw/64, transposed

        with (
            tc.tile_pool(name="early_b", bufs=1) as early_b,
            tc.tile_pool(name="scr_b", bufs=2) as scr_b,
            tc.tile_pool(name="mm_ps", bufs=4, space="PSUM") as mm_ps,
        ):
            qT = early_b.tile([P, 4, TC], F32, name="qT")
            kT = early_b.tile([P, 4, TC], F32, name="kT")
            v_sb = early_b.tile([P, 16, F], F32, name="v_sb")
            for wname, dstT, bsb, scb in (("wq", qT, bq_sb, scb_q),
                                          ("wk", kT, bk_sb, scb_k)):
                for a in range(4):
                    for n in range(4):
                        ps = mm_ps.tile([P, 512], F32, name="qkps")
                        for c in range(4):
                            nc.tensor.matmul(
                                ps[:], lhsT=w_sb[wname][:, c, a * P:(a + 1) * P],
                                rhs=xT[:, c, n * 512:(n + 1) * 512],
                                start=(c == 0), stop=(c == 3))
                        nc.scalar.activation(dstT[:, a, n * 512:(n + 1) * 512],
                                             ps[:], AF.Identity,
                                             bias=bsb[:, a:a + 1], scale=scb[:, :1])
            vals2 = tiny.tile([1, 2], F32, name="vals2")
            for idx, src in enumerate((qT, kT)):
                r1 = tiny.tile([P, 1], F32, name=f"qkmax{idx}")
                nc.vector.tensor_reduce(r1[:], src[:], axis=AX.XY, op=OP.max,
                                        apply_absolute_value=True)
                rm = cross_part_max(r1[:], f"qkgm{idx}")
                nc.vector.tensor_copy(vals2[:1, idx:idx + 1], rm[:1, :])
            ar2 = all_reduce(vals2[:1, :], 2, OP.max, "qk")

            # v projection (overlaps AR2qk)
            for i in range(16):
                ps = mm_ps.tile([P, 512], F32, name="qkps")
                for c in range(4):
                    nc.tensor.matmul(ps[:], lhsT=xT[:, c, i * P:(i + 1) * P],
                                     rhs=w_sb["wv"][:, c, :],
                                     start=(c == 0), stop=(c == 3))
                nc.vector.scalar_tensor_tensor(v_sb[:, i, :], ps[:], scb_v[:, :1],
                                               bv_rep[:], OP.mult, OP.add)
            vm1 = tiny.tile([P, 1], F32, name="vm1")
            nc.vector.tensor_reduce(vm1[:], v_sb[:], axis=AX.XY, op=OP.max,
                                    apply_absolute_value=True)
            vmg = cross_part_max(vm1[:], "vmg")
            ar2v = all_reduce(vmg[:1, :], 1, OP.max, "v")

            s2t = tiny.tile([1, 2], F32, name="s2t")
            nc.vector.tensor_scalar(s2t[:], ar2[:], 1.0 / QMAX, 1e-8, OP.mult, OP.max)
            r2t = tiny.tile([1, 2], F32, name="r2t")
            nc.vector.reciprocal(r2t[:], s2t[:])
            r_qb = bcast(r2t[:1, 0:1], "r_qb")
            r_kb = bcast(r2t[:1, 1:2], "r_kb")
            # alpha = s_q*s_k/8  (scores scale; /sqrt(64) == /8 exact)
            alpha = tiny.tile([1, 1], F32, name="alpha")
            nc.vector.tensor_tensor(alpha[:], s2t[:1, 0:1], s2t[:1, 1:2], OP.mult)
            nc.vector.tensor_scalar(alpha[:], alpha[:], 0.125, None, OP.mult)
            nalpha = tiny.tile([1, 1], F32, name="nalpha")
            nc.vector.tensor_scalar(nalpha[:], alpha[:], -1.0, None, OP.mult)
            ralpha = tiny.tile([1, 1], F32, name="ralpha")
            nc.vector.reciprocal(ralpha[:], alpha[:])
            alb = bcast(alpha[:1, :], "alb")
            nalb = bcast(nalpha[:1, :], "nalb")
            rab = bcast(ralpha[:1, :], "rab")

            for src, dst, rb in ((qT, qiT, r_qb), (kT, kiT, r_kb)):
                for c in range(4):
                    tq = scr_b.tile([P, TC], F32, name="tqnt")
                    nc.vector.tensor_scalar(tq[:], src[:, c], rb[:, :1], MAGIC_S,
                                            OP.mult, OP.add)
                    nc.vector.tensor_scalar(dst[:, c], tq[:], MAGIC_S, None,
                                            OP.subtract)
            if dbg:
                nc.sync.dma_start(dbg_qi.ap().rearrange("p (c t) -> p c t", c=4), qiT[:])
                nc.sync.dma_start(dbg_ki.ap().rearrange("p (c t) -> p c t", c=4), kiT[:])

            # fq(v) with AR2v result
            s_v, r_v = scale_of(ar2v[:1, :], "v")
            r_vb = bcast(r_v[:1, :], "r_vb")
            for c in range(4):
                tq = scr_b.tile([P, TC], F32, name="tqnt")
                nc.vector.tensor_scalar(tq[:], v_sb[:, 4 * c:4 * (c + 1), :],
                                        r_vb[:, :1], MAGIC_S, OP.mult, OP.add)
                nc.vector.tensor_scalar(vi[:, 4 * c:4 * (c + 1), :], tq[:],
                                        MAGIC_S, None, OP.subtract)

        # ============ stage 5: attention pass 1 (stats) ============
        with (
            tc.tile_pool(name="s_ps", bufs=2, space="PSUM") as s_ps,
            tc.tile_pool(name="escr", bufs=2) as escr,
        ):
            for b in range(BC):
                for h in range(H):
                    t = b * H + h
                    hp, hc = (h % 2) * 64, h // 2
                    ps = s_ps.tile([P, 4, 512], F32, name="sps")
                    for j in range(4):
                        nc.tensor.matmul(
                            ps[:, j, :],
                            lhsT=qiT[hp:hp + 64, hc,
                                     b * 512 + j * P: b * 512 + (j + 1) * P],
                            rhs=kiT[hp:hp + 64, hc, b * 512:(b + 1) * 512],
                            start=True, stop=True)
                    nc.vector.tensor_reduce(m_all[:, t * 4:(t + 1) * 4],
                                            ps[:], axis=AX.X, op=OP.max)
                    nc.vector.tensor_scalar(negm[:, t * 4:(t + 1) * 4],
                                            m_all[:, t * 4:(t + 1) * 4],
                                            nalb[:, :1], None, OP.mult)
                    for j in range(4):
                        es = escr.tile([P, 512], F32, name="es")
                        nc.scalar.activation(
                            es[:], ps[:, j, :], AF.Exp,
                            bias=negm[:, t * 4 + j:t * 4 + j + 1],
                            scale=alb[:, :1],
                            accum_out=z_all[:, t * 4 + j:t * 4 + j + 1])

            # Zmin -> AR3 -> s_a ; bias2 = -alpha*m - ln(Z*s_a)
            zr = tiny.tile([P, 1], F32, name="zr")
            nc.vector.tensor_reduce(zr[:], z_all[:], axis=AX.X, op=OP.min)
            nzr = tiny.tile([P, 1], F32, name="nzr")
            nc.vector.tensor_scalar(nzr[:], zr[:], -1.0, None, OP.mult)
            nzg = cross_part_max(nzr[:], "nzg")
            ar3 = all_reduce(nzg[:1, :], 1, OP.max, "zmin")
            zmin = tiny.tile([1, 1], F32, name="zmin")
            nc.vector.tensor_scalar(zmin[:], ar3[:], -1.0, None, OP.mult)
            amax = tiny.tile([1, 1], F32, name="amax")
            nc.vector.reciprocal(amax[:], zmin[:])
            s_a = tiny.tile([1, 1], F32, name="s_a")
            nc.vector.tensor_scalar(s_a[:], amax[:], 1.0 / QMAX, 1e-8,
                                    OP.mult, OP.max)
            sab = bcast(s_a[:1, :], "sab")
            zs = tiny.tile([P, P], F32, name="zs")
            nc.vector.tensor_scalar(zs[:], z_all[:], sab[:, :1], None, OP.mult)
            lnzs = tiny.tile([P, P], F32, name="lnzs")
            nc.scalar.activation(lnzs[:], zs[:], AF.Ln, bias=0.0, scale=1.0)
            # mw = (m + ln(Z*s_a)/alpha)/64, split into an exact fp16 cascade
            nc.vector.scalar_tensor_tensor(mw[:], lnzs[:], rab[:, :1], m_all[:],
                                           OP.mult, OP.add)
            nc.vector.tensor_scalar(mw[:], mw[:], 1.0 / 64.0, None, OP.mult)
            d1 = tiny.tile([P, P], FP16, name="d1")
            d2 = tiny.tile([P, P], FP16, name="d2")
            d3 = tiny.tile([P, P], FP16, name="d3")
            rr1 = tiny.tile([P, P], F32, name="rr1")
            rr2 = tiny.tile([P, P], F32, name="rr2")
            nc.vector.tensor_copy(d1[:], mw[:])
            nc.vector.tensor_tensor(rr1[:], mw[:], d1[:], OP.subtract)
            nc.vector.tensor_copy(d2[:], rr1[:])
            nc.vector.tensor_tensor(rr2[:], rr1[:], d2[:], OP.subtract)
            nc.vector.tensor_copy(d3[:], rr2[:])
            if dbg:
                nc.sync.dma_start(dbg_mz.ap()[:, 0:P], m_all[:])
                nc.sync.dma_start(dbg_mz.ap()[:, P:2 * P], z_all[:])

        # transpose digits to [t*4+j, p] layout for per-tile staging DMAs
        identh = const.tile([P, P], FP16, name="identh")
        make_identity(nc, identh)
        with tc.tile_pool(name="dg_ps", bufs=3, space="PSUM") as dg_ps:
            for r, dsrc in enumerate((d1, d2, d3)):
                pt = dg_ps.tile([P, P], FP16, name="dgp")
                nc.tensor.transpose(pt[:], dsrc[:], identh[:])
                nc.scalar.activation(digT[:, r, :], pt[:], AF.Identity,
                                     bias=0.0, scale=1.0)

        # ==== stage 6: pass 2 (aug K=67 matmul -> attn^T/s_a direct) + AV ====
        ctxp = stack.enter_context(tc.tile_pool(name="ctxp", bufs=1))
        ctxT = ctxp.tile([P, 4, TC], F32, name="ctxT")       # ctx^T [fi, tok]
        ctxiT = ctxp.tile([P, 4, TC], FP16, name="ctxiT")
        sav = tiny.tile([1, 1], F32, name="sav")
        nc.vector.tensor_tensor(sav[:], s_a[:], s_v[:], OP.mult)
        savb = bcast(sav[:1, :], "savb")
        with (
            tc.tile_pool(name="u_ps", bufs=3, space="PSUM") as u_ps,
            tc.tile_pool(name="c_ps", bufs=2, space="PSUM") as c_ps,
            tc.tile_pool(name="tf", bufs=2) as tf_pool,
            tc.tile_pool(name="ait", bufs=2) as ait_pool,
            tc.tile_pool(name="qstg", bufs=4) as qstg_pool,
            tc.tile_pool(name="kstg", bufs=4) as kstg_pool,
        ):
            ctx_ps = None
            for b in range(BC):
                for h in range(H):
                    t = b * H + h
                    hp, hc = (h % 2) * 64, h // 2
                    # stage augmented q~ / k~ [67, 512] fp16 tiles:
                    # rows 0-63 int grids, rows 64-66 the mw/64 cascade (q side)
                    # and -64 constants (k side).
                    qs = qstg_pool.tile([67, 512], FP16, name="qs")
                    ks = kstg_pool.tile([67, 512], FP16, name="ks")
                    nc.sync.dma_start(qs[0:64, :],
                                      qiT[hp:hp + 64, hc, b * 512:(b + 1) * 512])
                    nc.sync.dma_start(ks[0:64, :],
                                      kiT[hp:hp + 64, hc, b * 512:(b + 1) * 512])
                    nc.gpsimd.memset(ks[64:67, :], -64.0)
                    for r in range(3):
                        nc.sync.dma_start(qs[64 + r:65 + r, :],
                                          digT[t * 4:(t + 1) * 4, r, :])
                    ps2 = []
                    for half in range(2):
                        ph = u_ps.tile([P, 2, 512], F32, name="sps2")
                        ps2.append(ph)
                        for j2 in range(2):
                            jj = half * 2 + j2
                            nc.tensor.matmul(
                                ph[:, j2, :],
                                lhsT=ks[:, jj * P:(jj + 1) * P],
                                rhs=qs[:, :],
                                start=True, stop=True)
                    t_f32 = tf_pool.tile([P, 4, 512], F32, name="t_f32")
                    for jj in range(4):
                        nc.scalar.activation(
                            t_f32[:, jj, :], ps2[jj // 2][:, jj % 2, :], AF.Exp,
                            bias=0.0, scale=alb[:, :1])
                    ai_T = ait_pool.tile([P, 4, 512], FP16, name="ai_T")
                    nc.vector.tensor_scalar(ai_T[:], t_f32[:], MAGIC_U, MAGIC_U,
                                            OP.add, OP.subtract)
                    if dbg and t == 0:
                        nc.sync.dma_start(
                            dbg_ai.ap().rearrange("p (c t2) -> p c t2", c=4),
                            ai_T[:])
                    if h % 2 == 0:
                        ctx_ps = c_ps.tile([P, 512], F32, name="ctxps")
                    for jj in range(4):
                        nc.tensor.matmul(
                            ctx_ps[hp:hp + 64, :],
                            lhsT=vi[:, b * 4 + jj, h * 64:(h + 1) * 64],
                            rhs=ai_T[:, jj, :],
                            start=(jj == 0), stop=(jj == 3))
                    if h % 2 == 1:
                        nc.scalar.activation(ctxT[:, hc, b * 512:(b + 1) * 512],
                                             ctx_ps[:], AF.Identity,
                                             bias=0.0, scale=savb[:, :1])

            # ctx absmax -> AR4 -> fq(ctx)
            cm1 = tiny.tile([P, 1], F32, name="cm1")
            nc.vector.tensor_reduce(cm1[:], ctxT[:], axis=AX.XY, op=OP.max,
                                    apply_absolute_value=True)
            cmg = cross_part_max(cm1[:], "cmg")
            ar4 = all_reduce(cmg[:1, :], 1, OP.max, "ctx")
            s_c, r_c = scale_of(ar4[:1, :], "c")
            r_cb = bcast(r_c[:1, :], "r_cb")
            for c in range(4):
                tq = tf_pool.tile([P, TC], F32, name="ctxq")
                nc.vector.tensor_scalar(tq[:], ctxT[:, c], r_cb[:, :1], MAGIC_S,
                                        OP.mult, OP.add)
                nc.vector.tensor_scalar(ctxiT[:, c], tq[:], MAGIC_S, None,
                                        OP.subtract)
            if dbg:
                nc.sync.dma_start(dbg_ci.ap().rearrange("p (c t) -> p c t", c=4),
                                  ctxiT[:])

        # ============ stage 7: output projection + final fq ============
        sc_o = tiny.tile([1, 1], F32, name="sc_o")
        nc.vector.tensor_tensor(sc_o[:], s_c[:], wscl[:1, 3:4], OP.mult)
        scb_o = bcast(sc_o[:1, :], "scb_o")
        with (
            tc.tile_pool(name="outp", bufs=1) as outp,
            tc.tile_pool(name="o_ps", bufs=4, space="PSUM") as o_ps,
            tc.tile_pool(name="oscr", bufs=2) as oscr,
        ):
            out_sb = outp.tile([P, 16, F], F32, name="out_sb")
            for i in range(16):
                ps = o_ps.tile([P, 512], F32, name="ops")
                for c in range(4):
                    nc.tensor.matmul(ps[:], lhsT=ctxiT[:, c, i * P:(i + 1) * P],
                                     rhs=w_sb["wo"][:, c, :],
                                     start=(c == 0), stop=(c == 3))
                nc.vector.scalar_tensor_tensor(out_sb[:, i, :], ps[:], scb_o[:, :1],
                                               bo_rep[:], OP.mult, OP.add)
            om1 = tiny.tile([P, 1], F32, name="om1")
            nc.vector.tensor_reduce(om1[:], out_sb[:], axis=AX.XY, op=OP.max,
                                    apply_absolute_value=True)
            omg = cross_part_max(om1[:], "omg")
            ar5 = all_reduce(omg[:1, :], 1, OP.max, "out")
            s_o, r_o = scale_of(ar5[:1, :], "o")
            r_ob = bcast(r_o[:1, :], "r_ob")
            s_ob = bcast(s_o[:1, :], "s_ob")
            for c in range(4):
                tq = oscr.tile([P, TC], F32, name="outq")
                nc.vector.tensor_scalar(tq[:], out_sb[:, 4 * c:4 * (c + 1), :],
                                        r_ob[:, :1], MAGIC_S, OP.mult, OP.add)
                nc.vector.tensor_scalar(out_sb[:, 4 * c:4 * (c + 1), :], tq[:],
                                        MAGIC_S, s_ob[:, :1], OP.subtract, OP.mult)
            nc.sync.dma_start(out_d.ap().rearrange("(n p) f -> p n f", p=P),
                              out_sb[:])
            if dbg:
                ds = tiny.tile([1, 8], F32, name="ds")
                for i, src in enumerate((s_x[:1, :], s2t[:1, 0:1], s2t[:1, 1:2],
                                         s_v[:1, :], s_a[:1, :], s_c[:1, :],
                                         s_o[:1, :], alpha[:1, :])):
                    nc.vector.tensor_copy(ds[:1, i:i + 1], src)
                nc.sync.dma_start(dbg_scl.ap(), ds[:])

    nc.compile()
    return nc


def _fq_np(w):
    """Host-side per-tensor int8 fake-quant (matches reference fq in fp32)."""
    w = np.asarray(w, np.float32)
    s = np.maximum(np.abs(w).max() / np.float32(QMAX), np.float32(1e-8))
    wi = np.clip(np.round(w / s), -128, 127).astype(np.float32)
    return wi, np.float32(s)


def kernel(input_tensor, sequence_mask, ln_gamma, ln_beta,
           Wq, bq, Wk, bk, Wv, bv, Wo, bo):
    input_tensor = np.asarray(input_tensor, np.float32)
    assert np.asarray(sequence_mask).all(), "kernel specialized for all-true mask"
    assert np.all(np.asarray(ln_gamma) == 1.0) and np.all(np.asarray(ln_beta) == 0.0), \
        "kernel specialized for identity layernorm affine"

    if "nc" not in _CACHE:
        _CACHE["nc"] = _build()
    nc = _CACHE["nc"]

    wmaps = {}
    wscl = np.zeros((1, 4), np.float32)
    for i, (name, w) in enumerate((("wq", Wq), ("wk", Wk), ("wv", Wv), ("wo", Wo))):
        wi, s = _fq_np(w)
        wmaps[name] = np.ascontiguousarray(wi.T).astype(np.float16)
        wscl[0, i] = s
    biases = {"bq": bq, "bk": bk, "bv": bv, "bo": bo}

    in_maps = []
    for core in range(NCORES):
        m = {"x": np.ascontiguousarray(
                 input_tensor[core * BC:(core + 1) * BC].reshape(TC, F)),
             "wscl": wscl}
        m.update(wmaps)
        for n, v in biases.items():
            m[n] = np.ascontiguousarray(np.asarray(v, np.float32))
        in_maps.append(m)

    res = run_bass_kernel_spmd(nc, in_maps, core_ids=list(range(NCORES)),
                               **_CACHE.get("run_kwargs", {}))
    _CACHE["last_result"] = res
    out = np.concatenate([r["out"].reshape(BC, T, F) for r in res.results], axis=0)
    return out
